# revision 12
# baseline (speedup 1.0000x reference)
"""Locally-connected conv (LocalLinear) Trainium2 Bass kernel.

Problem: x (B=64, Cin=64, 32, 32), weight (Cout=64, Cin=64, 32, 32, 3, 3),
bias (Cout=64, 32, 32) -> out (B=64, Cout=64, 32, 32).
out[b,o,y,x] = sum_{c,u,v} xpad[b,c,y+u-1,x+v-1] * W[o,c,y,x,u,v] + bias[o,y,x]

Sharding: spatial rows across 8 cores (core i owns output rows y in
[4i, 4i+4) -> 128 locations/core).  Per location it's an independent
64x64 matmul with contraction 576 = Cin*9.

Compute scheme (tap t = 3u+v): all matmuls are K=128 with row
tile_position 0 (HW requires a constant row position within a PSUM
accumulation group; K=128-only keeps every group uniform).
  - xs0 SBUF partitions 0-63 hold x (with halo) for channel c=p;
    partitions 64-127 hold x shifted one window-COLUMN left:
    upper[r, cx] = lower[r, cx+1].  A K=128 matmul reading index (r, cx)
    contracts tap t=(u,v) on the lower half and t+1=(u,v+1) on the upper
    half -> tap pairs (0,1), (3,4), (6,7).
  - xs1 (rows 0-3 only) holds x on partitions 0-63 and x shifted one
    window-ROW up on partitions 64-127: upper[r, cx] = lower[r+1, cx]
    -> tap pair (2,5).  Built on-chip from xs0 with SBUF->SBUF copies
    on the scalar ring (no extra HBM traffic).
  - tap 8 is a K=128 matmul whose stationary upper 64 rows are zero.
  - locations are paired in the stationary columns: two x-adjacent
    locations (xA=2*xp, xB=2*xp+1) use PE col-groups 0/64, accumulating
    into psum partitions 0-63 / 64-127 of one bank (two sequential
    accumulation groups; 5 matmuls each).
  - matmul inputs fp16; PSUM fp32; bias added in the single drain op per
    location pair (DVE); output stored fp16, upcast to fp32 on host.
  - inputs stream on the sync HWDGE ring interleaved (xs rows / weight
    blocks) so the first matmul's deps (~2.9 MB) arrive early; xs1
    copies + output DMAs ride the scalar HWDGE ring.
"""

import numpy as np

import concourse.bacc as bacc
import concourse.mybir as mybir
import concourse.tile as tile
from concourse.bass_utils import run_bass_kernel_spmd

NCORES = 8
B = 64
CIN = 64
COUT = 64
H = 32
NJ = 64        # loc-pairs per core (4 yy rows x 16 xp)
JB = 8         # loc-pairs per weight DMA block
WF = 512       # weight cols per pair: 4 K=128 blocks x 128 (g,o)
OUT_G = 16     # loc-pairs per output DMA

F16 = mybir.dt.float16
F32 = mybir.dt.float32

_nc_cache = None


def _build_nc():
    from contextlib import ExitStack

    nc = bacc.Bacc("TRN2", target_bir_lowering=False)

    w_d = nc.dram_tensor("w", [128, NJ, WF], F16, kind="ExternalInput")
    w8_d = nc.dram_tensor("w8", [64, NJ, 128], F16, kind="ExternalInput")
    # xs rows 0-5: var0 (lower = x+halo, upper = shifted one col);
    # rows 6-9: var1 rows 0-3 (lower = x rows r, upper = x rows r+1).
    xs_d = nc.dram_tensor("xs", [128, 10, 35, B], F16, kind="ExternalInput")
    b_d = nc.dram_tensor("bias_p", [128, NJ], F32, kind="ExternalInput")
    o_d = nc.dram_tensor("out_p", [128, NJ, B], F16, kind="ExternalOutput")

    with tile.TileContext(nc) as tc, ExitStack() as ctx:
        xpool = ctx.enter_context(tc.tile_pool(name="xpool", bufs=1))
        wpool = ctx.enter_context(tc.tile_pool(name="wpool", bufs=1))
        bpool = ctx.enter_context(tc.tile_pool(name="bpool", bufs=1))
        opool = ctx.enter_context(tc.tile_pool(name="opool", bufs=1))
        pspool = ctx.enter_context(tc.tile_pool(name="ps", bufs=8, space="PSUM"))

        xs0 = xpool.tile([128, 10, 35, B], F16)
        w_sb = wpool.tile([128, NJ, WF], F16)
        w8_sb = wpool.tile([64, NJ, 128], F16)
        bias_sb = bpool.tile([128, NJ], F32)
        out_sb = opool.tile([128, NJ, B], F16)

        def xrows(eng, rows):
            for r in rows:
                eng.dma_start(xs0[:, r], xs_d[:, r])

        def wblk(eng, b):
            eng.dma_start(w_sb[:, b * JB:(b + 1) * JB, :],
                          w_d[:, b * JB:(b + 1) * JB, :])

        # Two HWDGE rings stream concurrently; each ring is FIFO, so
        # order within a ring = prefetch priority.  Ring transfer bubbles
        # on one ring are hidden by the other ring's transfers.
        nc.scalar.dma_start(bias_sb[:], b_d[:])
        xrows(nc.sync, (0, 1))
        nc.scalar.dma_start(w8_sb[:], w8_d[:])
        xrows(nc.sync, (2, 6))
        wblk(nc.scalar, 1)
        wblk(nc.sync, 0)
        xrows(nc.sync, (3, 7))
        wblk(nc.scalar, 3)
        wblk(nc.sync, 2)
        xrows(nc.sync, (4, 8))
        wblk(nc.scalar, 5)
        wblk(nc.sync, 4)
        xrows(nc.sync, (5, 9))
        wblk(nc.scalar, 7)
        wblk(nc.sync, 6)

        for j in range(NJ):
            yy, xp = divmod(j, 16)
            ps = pspool.tile([128, B], F32)
            # col-group g: loc x = 2*xp+g -> psum partitions 64g..64g+63.
            # Each group: 5 uniform K=128 matmuls at row position 0.
            for g in (0, 1):
                xloc = 2 * xp + g
                co = 64 * g
                ksl = slice(co, co + 64)
                nc.tensor.matmul(  # taps 0+1
                    ps[ksl, :], w_sb[0:128, j, co:co + 64],
                    xs0[0:128, yy + 0, xloc + 0, :],
                    start=True, stop=False, tile_position=(0, co))
                nc.tensor.matmul(  # taps 3+4
                    ps[ksl, :], w_sb[0:128, j, 128 + co:128 + co + 64],
                    xs0[0:128, yy + 1, xloc + 0, :],
                    start=False, stop=False, tile_position=(0, co))
                nc.tensor.matmul(  # taps 6+7
                    ps[ksl, :], w_sb[0:128, j, 256 + co:256 + co + 64],
                    xs0[0:128, yy + 2, xloc + 0, :],
                    start=False, stop=False, tile_position=(0, co))
                nc.tensor.matmul(  # taps 2+5 (row-shifted variant rows 6-9)
                    ps[ksl, :], w_sb[0:128, j, 384 + co:384 + co + 64],
                    xs0[0:128, 6 + yy, xloc + 2, :],
                    start=False, stop=False, tile_position=(0, co))
                nc.tensor.matmul(  # tap 8 (K=64, lower half)
                    ps[ksl, :], w8_sb[0:64, j, co:co + 64],
                    xs0[0:64, yy + 2, xloc + 2, :],
                    start=False, stop=True, tile_position=(0, co))
            # Single drain+bias op per pair.
            nc.vector.tensor_scalar_add(
                out_sb[:, j, :], ps[:], bias_sb[:, j:j + 1])
            if j % OUT_G == OUT_G - 1:
                j0 = j - (OUT_G - 1)
                nc.scalar.dma_start(
                    o_d[:, j0:j + 1, :], out_sb[:, j0:j + 1, :])

    nc.compile()
    return nc


def get_nc():
    global _nc_cache
    if _nc_cache is None:
        _nc_cache = _build_nc()
    return _nc_cache


def prep_inputs(x, weight, bias):
    """Host-side resharding/relayout -> list of 8 per-core input dicts."""
    x = np.asarray(x, dtype=np.float32)
    weight = np.asarray(weight, dtype=np.float32)
    bias = np.asarray(bias, dtype=np.float32)

    # x with halo: row slot = gy+1 (gy in -1..32), col slot = gx+1
    # (gx in -1..33; slot 34 == gx 33 is zero padding for the shifted
    # upper half).  Core i sees rows gy = 4i-1 .. 4i+4 (slots 4i..4i+5).
    xpad = np.zeros((B, CIN, H + 2, H + 3), np.float32)
    xpad[:, :, 1:H + 1, 1:H + 1] = x
    xs = np.zeros((NCORES, 128, 10, H + 3, B), np.float16)
    for i in range(NCORES):
        s = xpad[:, :, 4 * i:4 * i + 6, :].transpose(1, 2, 3, 0)  # (c,6,35,b)
        xs[i, 0:64, 0:6] = s
        xs[i, 64:128, 0:6, 0:H + 2, :] = s[:, :, 1:H + 3, :]
        xs[i, 0:64, 6:10] = s[:, 0:4]     # var1 lower: rows r
        xs[i, 64:128, 6:10] = s[:, 1:5]   # var1 upper: rows r+1

    # weights: w[i, p, j=(yy,xp), f]; four 128-col K=128 blocks per pair
    # (f = 128k + 64g + o): k=0..2 tap pairs (0,1),(3,4),(6,7) [lower tap
    # on partitions 0-63, upper on 64-127]; k=3 taps (2,5).  Tap 8 lives
    # in w8 [64, NJ, (g,o)].
    Wr = weight.reshape(COUT, CIN, NCORES, 4, 16, 2, 9)  # o c i yy xp g t
    lo = Wr[..., [0, 3, 6, 2]]                           # o c i yy xp g k
    up = Wr[..., [1, 4, 7, 5]]
    wlo = lo.transpose(2, 1, 3, 4, 6, 5, 0).reshape(NCORES, CIN, NJ, WF)
    wup = up.transpose(2, 1, 3, 4, 6, 5, 0).reshape(NCORES, CIN, NJ, WF)
    wp = np.empty((NCORES, 128, NJ, WF), np.float16)
    wp[:, 0:64] = wlo
    wp[:, 64:128] = wup
    w8p = np.ascontiguousarray(
        Wr[..., 8].transpose(2, 1, 3, 4, 5, 0).reshape(NCORES, CIN, NJ, 128),
        dtype=np.float16)

    # bias: bp[i, p=(g,o), j]
    Bb = bias.reshape(COUT, NCORES, 4, 16, 2)  # o i yy xp g
    bp = np.ascontiguousarray(
        Bb.transpose(1, 4, 0, 2, 3).reshape(NCORES, 128, NJ), dtype=np.float32)

    return [
        {"w": np.ascontiguousarray(wp[i]),
         "w8": w8p[i],
         "xs": np.ascontiguousarray(xs[i]),
         "bias_p": bp[i]}
        for i in range(NCORES)
    ]


def unpack_output(results):
    """results: list of 8 dicts with 'out_p' [128, NJ, B] -> (B, COUT, H, H)."""
    allout = np.stack([np.asarray(r["out_p"], np.float32) for r in results])
    a = allout.reshape(NCORES, 2, COUT, 4, 16, B)     # i g o yy xp b
    out = a.transpose(5, 2, 0, 3, 4, 1).reshape(B, COUT, H, H)
    return np.ascontiguousarray(out, dtype=np.float32)


def kernel(x, weight, bias, _trace=False, _tmpdir=None):
    nc = get_nc()
    in_maps = prep_inputs(x, weight, bias)
    res = run_bass_kernel_spmd(
        nc, in_maps, core_ids=list(range(NCORES)),
        trace=_trace, tmpdir=_tmpdir,
        **({"trace_cores": list(range(NCORES))} if _trace else {}),
    )
    out = unpack_output(res.results)
    if _trace:
        kernel.last_results = res
    return out


# revision 19
# speedup vs baseline: 1.4430x; 1.4430x over previous
"""Locally-connected conv (LocalLinear) Trainium2 Bass kernel.

Problem: x (B=64, Cin=64, 32, 32), weight (Cout=64, Cin=64, 32, 32, 3, 3),
bias (Cout=64, 32, 32) -> out (B=64, Cout=64, 32, 32).
out[b,o,y,x] = sum_{c,u,v} xpad[b,c,y+u-1,x+v-1] * W[o,c,y,x,u,v] + bias[o,y,x]

Sharding: spatial rows across 8 cores (core i owns output rows y in
[4i, 4i+4) -> 128 locations/core).  Per location it's an independent
64x64 matmul with contraction 576 = Cin*9.

Compute scheme (tap t = 3u+v): all matmuls are K=128 with row
tile_position 0 (HW requires a constant row position within a PSUM
accumulation group; K=128-only keeps every group uniform).
  - xs0 SBUF partitions 0-63 hold x (with halo) for channel c=p;
    partitions 64-127 hold x shifted one window-COLUMN left:
    upper[r, cx] = lower[r, cx+1].  A K=128 matmul reading index (r, cx)
    contracts tap t=(u,v) on the lower half and t+1=(u,v+1) on the upper
    half -> tap pairs (0,1), (3,4), (6,7).
  - xs1 (rows 0-3 only) holds x on partitions 0-63 and x shifted one
    window-ROW up on partitions 64-127: upper[r, cx] = lower[r+1, cx]
    -> tap pair (2,5).  Built on-chip from xs0 with SBUF->SBUF copies
    on the scalar ring (no extra HBM traffic).
  - tap 8 is a K=128 matmul whose stationary upper 64 rows are zero.
  - locations are paired in the stationary columns: two x-adjacent
    locations (xA=2*xp, xB=2*xp+1) use PE col-groups 0/64, accumulating
    into psum partitions 0-63 / 64-127 of one bank (two sequential
    accumulation groups; 5 matmuls each).
  - matmul inputs fp16; PSUM fp32; bias added in the single drain op per
    location pair (DVE); output stored fp16, upcast to fp32 on host.
  - inputs stream on the sync HWDGE ring interleaved (xs rows / weight
    blocks) so the first matmul's deps (~2.9 MB) arrive early; xs1
    copies + output DMAs ride the scalar HWDGE ring.
"""

import numpy as np

import concourse.bacc as bacc
import concourse.mybir as mybir
import concourse.tile as tile
from concourse.bass_utils import run_bass_kernel_spmd

NCORES = 8
B = 64
CIN = 64
COUT = 64
H = 32
NJ = 64        # loc-pairs per core (4 yy rows x 16 xp)
JB = 8         # loc-pairs per weight DMA block
WF = 640       # weight cols per pair: 5 K=128 blocks x 128 (g,o)
OUT_G = 16     # loc-pairs per output DMA

F16 = mybir.dt.float16
F32 = mybir.dt.float32

_nc_cache = None


def _build_nc():
    from contextlib import ExitStack

    nc = bacc.Bacc("TRN2", target_bir_lowering=False)

    w_d = nc.dram_tensor("w", [128, NJ, WF], F16, kind="ExternalInput")
    xs_d = nc.dram_tensor("xs", [128, 6, 35, B], F16, kind="ExternalInput")
    b_d = nc.dram_tensor("bias_p", [128, NJ], F32, kind="ExternalInput")
    o_d = nc.dram_tensor("out_p", [128, NJ, B], F16, kind="ExternalOutput")

    with tile.TileContext(nc) as tc, ExitStack() as ctx:
        xpool = ctx.enter_context(tc.tile_pool(name="xpool", bufs=1))
        wpool = ctx.enter_context(tc.tile_pool(name="wpool", bufs=1))
        bpool = ctx.enter_context(tc.tile_pool(name="bpool", bufs=1))
        opool = ctx.enter_context(tc.tile_pool(name="opool", bufs=1))
        pspool = ctx.enter_context(tc.tile_pool(name="ps", bufs=8, space="PSUM"))

        xs0 = xpool.tile([128, 6, 35, B], F16)
        xs1 = xpool.tile([128, 4, 35, B], F16)
        w_sb = wpool.tile([128, NJ, WF], F16)
        bias_sb = bpool.tile([128, NJ], F32)
        out_sb = opool.tile([128, NJ, B], F16)

        def wblk(eng, b):
            eng.dma_start(w_sb[:, b * JB:(b + 1) * JB, :],
                          w_d[:, b * JB:(b + 1) * JB, :])

        def xs1row(eng, r):
            # xs1 lower r <- xs0 lower r; xs1 upper r <- xs0 lower r+1
            eng.dma_start(xs1[0:64, r], xs0[0:64, r])
            eng.dma_start(xs1[64:128, r], xs0[0:64, r + 1])

        # Two HWDGE rings stream concurrently; each ring is FIFO, so
        # order within a ring = prefetch priority.  Ring transfer bubbles
        # on one ring are hidden by the other ring's transfers.  The
        # SBUF->SBUF xs1 copies (fabric, not HBM) ride the scalar ring.
        nc.sync.dma_start(xs0[:, 0:2], xs_d[:, 0:2])
        nc.scalar.dma_start(bias_sb[:], b_d[:])
        wblk(nc.sync, 0)
        wblk(nc.scalar, 1)
        nc.sync.dma_start(xs0[:, 2:4], xs_d[:, 2:4])
        xs1row(nc.scalar, 0)
        wblk(nc.sync, 2)
        wblk(nc.scalar, 3)
        nc.sync.dma_start(xs0[:, 4:6], xs_d[:, 4:6])
        xs1row(nc.scalar, 1)
        xs1row(nc.scalar, 2)
        wblk(nc.sync, 4)
        wblk(nc.scalar, 5)
        xs1row(nc.scalar, 3)
        wblk(nc.sync, 6)
        wblk(nc.scalar, 7)

        for j in range(NJ):
            yy, xp = divmod(j, 16)
            ps = pspool.tile([128, B], F32)
            # col-group g: loc x = 2*xp+g -> psum partitions 64g..64g+63.
            # Each group: 5 uniform K=128 matmuls at row position 0.
            for g in (0, 1):
                xloc = 2 * xp + g
                co = 64 * g
                ksl = slice(co, co + 64)
                nc.tensor.matmul(  # taps 0+1
                    ps[ksl, :], w_sb[0:128, j, co:co + 64],
                    xs0[0:128, yy + 0, xloc + 0, :],
                    start=True, stop=False, tile_position=(0, co))
                nc.tensor.matmul(  # taps 3+4
                    ps[ksl, :], w_sb[0:128, j, 128 + co:128 + co + 64],
                    xs0[0:128, yy + 1, xloc + 0, :],
                    start=False, stop=False, tile_position=(0, co))
                nc.tensor.matmul(  # taps 6+7
                    ps[ksl, :], w_sb[0:128, j, 256 + co:256 + co + 64],
                    xs0[0:128, yy + 2, xloc + 0, :],
                    start=False, stop=False, tile_position=(0, co))
                nc.tensor.matmul(  # taps 2+5 (row-shifted variant)
                    ps[ksl, :], w_sb[0:128, j, 384 + co:384 + co + 64],
                    xs1[0:128, yy, xloc + 2, :],
                    start=False, stop=False, tile_position=(0, co))
                nc.tensor.matmul(  # tap 8 (stationary upper rows zero)
                    ps[ksl, :], w_sb[0:128, j, 512 + co:512 + co + 64],
                    xs0[0:128, yy + 2, xloc + 2, :],
                    start=False, stop=True, tile_position=(0, co))
            # Single drain+bias op per pair.
            nc.vector.tensor_scalar_add(
                out_sb[:, j, :], ps[:], bias_sb[:, j:j + 1])
            if j % OUT_G == OUT_G - 1:
                j0 = j - (OUT_G - 1)
                # Alternate output DMAs across rings; the input streams
                # have mostly drained by the time these fire.
                eng = nc.sync if (j // OUT_G) % 2 == 0 else nc.scalar
                eng.dma_start(o_d[:, j0:j + 1, :], out_sb[:, j0:j + 1, :])

    nc.compile()
    return nc


def get_nc():
    global _nc_cache
    if _nc_cache is None:
        _nc_cache = _build_nc()
    return _nc_cache


def prep_inputs(x, weight, bias):
    """Host-side resharding/relayout -> list of 8 per-core input dicts."""
    x = np.asarray(x, dtype=np.float32)
    weight = np.asarray(weight, dtype=np.float32)
    bias = np.asarray(bias, dtype=np.float32)

    # x with halo: row slot = gy+1 (gy in -1..32), col slot = gx+1
    # (gx in -1..33; slot 34 == gx 33 is zero padding for the shifted
    # upper half).  Core i sees rows gy = 4i-1 .. 4i+4 (slots 4i..4i+5).
    xpad = np.zeros((B, CIN, H + 2, H + 3), np.float32)
    xpad[:, :, 1:H + 1, 1:H + 1] = x
    xs = np.zeros((NCORES, 128, 6, H + 3, B), np.float16)
    for i in range(NCORES):
        s = xpad[:, :, 4 * i:4 * i + 6, :].transpose(1, 2, 3, 0)  # (c,6,35,b)
        xs[i, 0:64] = s
        xs[i, 64:128, :, 0:H + 2, :] = s[:, :, 1:H + 3, :]

    # weights: w[i, p, j=(yy,xp), f]; five 128-col K=128 blocks per pair
    # (f = 128k + 64g + o): k=0..2 tap pairs (0,1),(3,4),(6,7) [lower tap
    # on partitions 0-63, upper on 64-127]; k=3 taps (2,5); k=4 tap 8
    # (upper rows zero).
    Wr = weight.reshape(COUT, CIN, NCORES, 4, 16, 2, 9)  # o c i yy xp g t
    lo = Wr[..., [0, 3, 6, 2, 8]]                        # o c i yy xp g k
    up = Wr[..., [1, 4, 7, 5]]
    wlo = lo.transpose(2, 1, 3, 4, 6, 5, 0).reshape(NCORES, CIN, NJ, WF)
    wup = up.transpose(2, 1, 3, 4, 6, 5, 0).reshape(NCORES, CIN, NJ, 512)
    wp = np.zeros((NCORES, 128, NJ, WF), np.float16)
    wp[:, 0:64] = wlo
    wp[:, 64:128, :, 0:512] = wup

    # bias: bp[i, p=(g,o), j]
    Bb = bias.reshape(COUT, NCORES, 4, 16, 2)  # o i yy xp g
    bp = np.ascontiguousarray(
        Bb.transpose(1, 4, 0, 2, 3).reshape(NCORES, 128, NJ), dtype=np.float32)

    return [
        {"w": np.ascontiguousarray(wp[i]),
         "xs": np.ascontiguousarray(xs[i]),
         "bias_p": bp[i]}
        for i in range(NCORES)
    ]


def unpack_output(results):
    """results: list of 8 dicts with 'out_p' [128, NJ, B] -> (B, COUT, H, H)."""
    allout = np.stack([np.asarray(r["out_p"], np.float32) for r in results])
    a = allout.reshape(NCORES, 2, COUT, 4, 16, B)     # i g o yy xp b
    out = a.transpose(5, 2, 0, 3, 4, 1).reshape(B, COUT, H, H)
    return np.ascontiguousarray(out, dtype=np.float32)


def kernel(x, weight, bias, _trace=False, _tmpdir=None):
    nc = get_nc()
    in_maps = prep_inputs(x, weight, bias)
    res = run_bass_kernel_spmd(
        nc, in_maps, core_ids=list(range(NCORES)),
        trace=_trace, tmpdir=_tmpdir,
        **({"trace_cores": list(range(NCORES))} if _trace else {}),
    )
    out = unpack_output(res.results)
    if _trace:
        kernel.last_results = res
    return out
